# revision 10
# baseline (speedup 1.0000x reference)
"""Trainium2 Bass kernel for nn_AttentionModule (B=4, C=256, 64x64 spatial).

Reference computation (per batch b, x flattened to [C, HW]):
    q = Wq @ x + bq            [32, HW]
    k = Wk @ x + bk            [32, HW]
    v = x^T @ Wv^T + bv        [HW, 256]
    out = softmax(q^T @ k) @ v [HW, 256] -> transposed to [C, HW]

Sharding: 8 cores, data-parallel over (batch, query-half): core = 2*b + h
computes queries [h*2048, (h+1)*2048) of batch b against all 4096 keys.
Weights replicated; the per-core q slice arrives as separate input data
(xq) so the program stays SPMD-identical.

Numerics: fp16 inputs/projections, fp32 PSUM accumulation, bf16 attention
probabilities (fp16 would overflow: scores reach +-39). ~4.8e-3 max-rel
vs the fp32 reference.

Device design:
  - scores computed transposed ([keys, q]) so the softmax denominator is
    accumulated by the PE itself: v carries ones columns; out[:, 256] =
    sum_k exp(s). exp on ScalarE straight out of PSUM (no max-subtract).
  - QK is 2-way row-packed: even k tiles at partitions 0-31, odd at
    32-63 (PE row groups), q replicated to both blocks. Pair g scores
    k tiles (2g, 2g+1) into one [128, 1024] PSUM span -> one ACTIVATE.
  - projections write both partition blocks directly via column-tiled
    duplicate matmuls (tile_position=(0,32)) - no cross-partition moves.
  - flat 64-pair software pipeline: QK/exp stream, projections ride the
    first ~13 steps (PSUM "o" slots are proj-owned until AV starts),
    then AV catches up at 2 pairs/step and settles to a 2-pair lag.
  - AV accumulates [q, 258] in PSUM; normalization is a per-partition
    reciprocal + tensor_scalar multiply on VectorE; fp16 out tiles.
  - final [q, c] -> [c, q] transpose + bv add happen host-side in the
    unshard step.
"""
import numpy as np
from contextlib import ExitStack

import concourse.bass as bass
import concourse.bacc as bacc
import concourse.tile as tile
from concourse import mybir
from concourse.bass_utils import run_bass_kernel_spmd

B, C, H, W = 4, 256, 64, 64
HW = H * W            # 4096
D = C // 8            # 32 (q/k channels)
NCORES = 8
Q = HW // 2           # 2048 queries per core
QC = 512              # q chunk (matmul moving dim)
NCH = Q // QC         # 4 chunks
KT = HW // 128        # 32 key tiles
P = 128
VW = C + 2            # v tile width (ones col + even-pad)
WPK = 2 * D + C       # packed weight cols per half: wq|wk|wv

F32 = mybir.dt.float32
F16 = mybir.dt.float16
BF16 = mybir.dt.bfloat16
EXP = mybir.ActivationFunctionType.Exp

_CACHE: dict = {}


def build_program() -> bacc.Bacc:
    nc = bacc.Bacc("TRN2", target_bir_lowering=False, debug=False)

    xkv_d = nc.dram_tensor("xkv", [C, HW], F16, kind="ExternalInput").ap()
    xq_d = nc.dram_tensor("xq", [C, Q], F16, kind="ExternalInput").ap()
    # packed per c'-half: [wqT | wkT | wvT]  [256, 320]
    wpk_d = nc.dram_tensor("wpk", [C, WPK], F16, kind="ExternalInput").ap()
    # packed [bq | bk | ones(QC)]  [1, 64 + QC]
    bpk_d = nc.dram_tensor("bpk", [1, 2 * D + QC], F16, kind="ExternalInput").ap()
    o_d = nc.dram_tensor("o", [Q, C], F16, kind="ExternalOutput").ap()

    with tile.TileContext(nc) as tc:
        with ExitStack() as ctx:
            big = ctx.enter_context(tc.tile_pool(name="big", bufs=24))
            const = ctx.enter_context(tc.tile_pool(name="const", bufs=1))
            ep = ctx.enter_context(tc.tile_pool(name="ep", bufs=4))
            ps = ctx.enter_context(tc.tile_pool(name="ps", bufs=2, space="PSUM"))
            po = ctx.enter_context(tc.tile_pool(name="po", bufs=4, space="PSUM"))

            # ---- weights / biases: 3 quick triggers on the scalar queue ----
            wpk_t = [const.tile([P, WPK], F16, tag=f"wpk{i}", name=f"wpk{i}")
                     for i in range(2)]
            for i in range(2):
                nc.scalar.dma_start(wpk_t[i][:], wpk_d[i * P:(i + 1) * P, :])
            bpk_t = const.tile([1, 2 * D + QC], F16, tag="bpk")
            nc.scalar.dma_start(bpk_t[:], bpk_d)
            wq_sb = [wpk_t[i][:, 0:D] for i in range(2)]
            wk_sb = [wpk_t[i][:, D:2 * D] for i in range(2)]
            wv_sb = [wpk_t[i][:, 2 * D:WPK] for i in range(2)]
            bq_sb = bpk_t[:, 0:D]
            bk_sb = bpk_t[:, D:2 * D]
            ones_sb = bpk_t[:, 2 * D:]

            # qrep: q^T at partition blocks 0-31 and 32-63 (identical data)
            qrep = const.tile([2 * D, Q], F16, tag="qrep")
            # kT4: EVEN k tiles at partitions 0-31 (col (kt//2)*128),
            #      ODD  k tiles at partitions 32-63
            kT4 = const.tile([2 * D, 16 * P], F16, tag="kT4")
            v_all = const.tile([P, KT * VW], F16, tag="vall")
            nc.vector.memset(
                v_all[:].rearrange("p (k c) -> p k c", c=VW)[:, :, C:C + 2], 1.0)
            v_sb = [v_all[:, t * VW:(t + 1) * VW] for t in range(KT)]

            # ---- x tiles: sync queue for half 0, gpsimd (SWDGE) for half 1;
            # pieces ordered by first use ----
            xkv_t = [const.tile([P, HW], F16, tag=f"xkv{i}", name=f"xkv{i}")
                     for i in range(2)]
            xq_t = [const.tile([P, Q], F16, tag=f"xq{i}", name=f"xq{i}")
                    for i in range(2)]
            eng = [nc.sync, nc.gpsimd]
            for i in range(2):
                e = eng[i]
                e.dma_start(xkv_t[i][:, 0:QC], xkv_d[i * P:(i + 1) * P, 0:QC])
                e.dma_start(xq_t[i][:, 0:QC], xq_d[i * P:(i + 1) * P, 0:QC])
                e.dma_start(xkv_t[i][:, QC:2 * QC],
                            xkv_d[i * P:(i + 1) * P, QC:2 * QC])
                e.dma_start(xq_t[i][:, QC:], xq_d[i * P:(i + 1) * P, QC:])
                e.dma_start(xkv_t[i][:, 2 * QC:HW],
                            xkv_d[i * P:(i + 1) * P, 2 * QC:HW])
            xkv_sb = [[xkv_t[i][:, j * QC:(j + 1) * QC] for j in range(HW // QC)]
                      for i in range(2)]
            xq_sb = [[xq_t[i][:, j * QC:(j + 1) * QC] for j in range(Q // QC)]
                     for i in range(2)]

            # ---- projections: each chunk written to BOTH partition blocks
            # via column-tiled duplicate matmuls ----
            def dual_proj(dst, w0, w1, b, x0, x1):
                """dst [64, QC] psum gets w^T x + b at rows 0-31 AND 32-63."""
                for blk in range(2):
                    tp = (0, blk * D)
                    nc.tensor.matmul(dst[blk * D:(blk + 1) * D, :], w0, x0,
                                     start=True, stop=False, tile_position=tp)
                    nc.tensor.matmul(dst[blk * D:(blk + 1) * D, :], w1, x1,
                                     start=False, stop=False, tile_position=tp)
                    nc.tensor.matmul(dst[blk * D:(blk + 1) * D, :], b, ones_sb,
                                     start=False, stop=True, tile_position=tp)

            def kproj(j):
                kp = po.tile([2 * D, QC], F32, tag="o", name="kp")
                dual_proj(kp, wk_sb[0], wk_sb[1], bk_sb,
                          xkv_sb[0][j], xkv_sb[1][j])
                # chunk j holds k tiles 4j..4j+3; even -> block 0, odd -> block 1
                src = kp[:].rearrange("p (t c) -> p t c", c=P)
                dst = kT4[:, 2 * j * P:(2 * j + 2) * P].rearrange(
                    "p (t c) -> p t c", c=P)
                nc.vector.tensor_copy(dst[0:D], src[0:D, 0::2])
                nc.vector.tensor_copy(dst[D:2 * D], src[D:2 * D, 1::2])

            def qproj(j):
                qp = po.tile([2 * D, QC], F32, tag="o", name="qp")
                dual_proj(qp, wq_sb[0], wq_sb[1], bq_sb,
                          xq_sb[0][j], xq_sb[1][j])
                nc.vector.tensor_copy(qrep[:, j * QC:(j + 1) * QC], qp[:])

            def vproj(t):
                j, off = divmod(t, QC // P)
                vp = po.tile([P, C], F32, tag="o", name="vp")
                nc.tensor.matmul(
                    vp[:], xkv_sb[0][j][:, off * P:(off + 1) * P], wv_sb[0],
                    start=True, stop=False)
                nc.tensor.matmul(
                    vp[:], xkv_sb[1][j][:, off * P:(off + 1) * P], wv_sb[1],
                    start=False, stop=True)
                nc.vector.tensor_copy(v_sb[t][:, 0:C], vp[:])

            def vpair(g):
                vproj(2 * g)
                vproj(2 * g + 1)

            # chunk-0 critical deps first; the rest rides the early steps
            kproj(0)
            qproj(0)
            proj_work = []
            for j in (1, 2, 3):
                proj_work += [lambda j=j: kproj(j), lambda g=2 * (j - 1): vpair(g),
                              lambda g=2 * j - 1: vpair(g), lambda j=j: qproj(j)]
            for j in (4, 5, 6, 7):
                proj_work += [lambda j=j: kproj(j), lambda g=2 * (j - 1): vpair(g),
                              lambda g=2 * j - 1: vpair(g)]
            proj_work += [lambda: vpair(14), lambda: vpair(15)]

            # ---- attention: flat 64-pair stream ----
            # pair (ci, g) scores k tiles (2g, 2g+1) against q chunk ci.
            NP = NCH * 16
            AV_START = 15      # po banks are proj-owned before this step

            def av_epilogue(ops, ci):
                for qs in range(QC // P):
                    op = ops[qs]
                    rinv = ep.tile([P, 1], F32, tag="rinv", name="rinv")
                    nc.vector.reciprocal(rinv[:], op[:, C:C + 1])
                    osb = ep.tile([P, C], F16, tag="osb", name="osb")
                    nc.vector.tensor_scalar_mul(osb[:], op[:, 0:C], rinv[:])
                    q0 = (ci * (QC // P) + qs) * P
                    nc.sync.dma_start(o_d[q0:q0 + P, :], osb[:])

            def av_pair(ops, pair_tile, g):
                for half in range(2):
                    kt = 2 * g + half
                    for qs in range(QC // P):
                        nc.tensor.matmul(
                            ops[qs][:],
                            pair_tile[:, half * QC + qs * P: half * QC + (qs + 1) * P],
                            v_sb[kt][:],
                            start=(kt == 0), stop=(kt == KT - 1))

            pair_tiles = {}
            ops = {}
            av_done = 0
            step = 0
            wi = 0
            while av_done < NP:
                if step < NP:
                    ci, g = divmod(step, 16)
                    sc = ps.tile([P, 2 * QC], F32, tag="p", name="sc")
                    nc.tensor.matmul(
                        sc[:, 0:QC], kT4[0:D, g * P:(g + 1) * P],
                        qrep[0:D, ci * QC:(ci + 1) * QC],
                        start=True, stop=True, tile_position=(0, 0))
                    nc.tensor.matmul(
                        sc[:, QC:2 * QC], kT4[D:2 * D, g * P:(g + 1) * P],
                        qrep[D:2 * D, ci * QC:(ci + 1) * QC],
                        start=True, stop=True, tile_position=(D, 0))
                    Pt = big.tile([P, 2 * QC], BF16, tag="big", name="pt")
                    nc.scalar.activation(Pt[:], sc[:], EXP)
                    pair_tiles[step] = Pt
                for _ in range(2):
                    if wi < len(proj_work):
                        proj_work[wi]()
                        wi += 1
                budget = 2 if step >= AV_START else 0
                while budget > 0 and av_done < NP and av_done <= step - 2:
                    cav, gav = divmod(av_done, 16)
                    if gav == 0:
                        ops[cav] = [po.tile([P, VW], F32, tag="o", name="avo")
                                    for _ in range(QC // P)]
                    av_pair(ops[cav], pair_tiles.pop(av_done), gav)
                    if gav == 15:
                        av_epilogue(ops.pop(cav), cav)
                    av_done += 1
                    budget -= 1
                step += 1

    nc.compile()
    return nc


def _in_maps(x, Wq, bq, Wk, bk, Wv, bv):
    xf = np.ascontiguousarray(np.asarray(x, np.float32).reshape(B, C, HW)).astype(np.float16)
    wqT = np.asarray(Wq, np.float32).T
    wkT = np.asarray(Wk, np.float32).T
    wvT = np.asarray(Wv, np.float32).T
    wpk = np.concatenate([wqT, wkT, wvT], axis=1).astype(np.float16)
    bpk = np.concatenate([
        np.asarray(bq, np.float32).reshape(1, D),
        np.asarray(bk, np.float32).reshape(1, D),
        np.ones((1, QC), np.float32)], axis=1).astype(np.float16)
    maps = []
    for core in range(NCORES):
        b, h = divmod(core, 2)
        maps.append({
            "xkv": xf[b],
            "xq": np.ascontiguousarray(xf[b][:, h * Q:(h + 1) * Q]),
            "wpk": np.ascontiguousarray(wpk),
            "bpk": np.ascontiguousarray(bpk),
        })
    return maps


def _gather(results, bv):
    out = np.empty((B, C, HW), np.float32)
    for core in range(NCORES):
        b, h = divmod(core, 2)
        out[b][:, h * Q:(h + 1) * Q] = results[core]["o"].T
    out += np.asarray(bv, np.float32).reshape(1, C, 1)
    return out.reshape(B, C, H, W)


def run(x, Wq, bq, Wk, bk, Wv, bv, **kwargs):
    nc = _CACHE.get("nc")
    if nc is None:
        nc = build_program()
        _CACHE["nc"] = nc
    maps = _in_maps(x, Wq, bq, Wk, bk, Wv, bv)
    res = run_bass_kernel_spmd(nc, maps, core_ids=list(range(NCORES)), **kwargs)
    return _gather(res.results, bv), res


def kernel(x, Wq, bq, Wk, bk, Wv, bv) -> np.ndarray:
    out, _ = run(x, Wq, bq, Wk, bk, Wv, bv)
    return out
